# revision 1
# baseline (speedup 1.0000x reference)
"""ConvGAU (gated attention unit with 1x1 conv projections) on 8 TRN2 NeuronCores.

Data-parallel: B=16 images sharded 2-per-core across 8 cores; every op is
batch-independent so there is no cross-core communication.

Per-image compute (C=256, N=48*48=2304, HID=512, QK=96), all matmuls in
float32r (fp32 storage, FP22-truncated reads -> full-rate on the PE):

  q,k  = silu(w_qk x + b_qk)          layout [96, N]   (qk-channels on partitions)
  vT   = silu(x^T w_v^T + b_v)        layout [N, 512]  (positions on partitions)
  gate = silu(w_g x + b_g)            layout [512, N]
  per n-chunk (512 cols):
    simT_j = k_j^T q_chunk            [128, S] PSUM   (j = 18 chunks of 128 positions)
    AT_j   = relu(simT_j)^2           one DVE scalar_tensor_tensor: (x max 0) * x
    V[hs] += vT_j[:,hs]^T @ AT_j      PSUM accumulate over j   [128, S] x 4
    Vg[hs] = V[hs] * gate[hs, chunk]  DVE
    out[os] = sum_hs w_oT[hs,os]^T @ Vg[hs]; (out + b_out) + x  one DVE; DMA out
"""

import numpy as np
from contextlib import ExitStack

import concourse.bass as bass
import concourse.tile as tile
from concourse import bacc
from concourse import mybir
from concourse.bass_utils import run_bass_kernel_spmd

B, C, N = 16, 256, 48 * 48
HID, QK = 512, 96
NCORES = 8
BPC = B // NCORES  # images per core

F32 = mybir.dt.float32
F32R = mybir.dt.float32r
AF = mybir.ActivationFunctionType
ALU = mybir.AluOpType

# n-chunks of the 2304 spatial positions (free dim of most matmuls; >=256 keeps
# float32r at full rate, <=512 fits one PSUM bank)
NCH = [(0, 512), (512, 512), (1024, 512), (1536, 512), (2048, 256)]
NJ = N // 128  # 18 key/position chunks


def _r(ap):
    return ap.bitcast(F32R)


def build_bass(with_bv: bool, act=None, reps: int = 1) -> bass.Bass:
    if act is None:
        act = AF.Silu
    nc = bacc.Bacc("TRN2", target_bir_lowering=False, debug=False)

    # packed weights (host-side prep): w_qk [C, 192] = [w_qT | w_kT];
    # w_vg [C, 1024] = [w_vT | w_gT]; w_op [128, 1024] = 4 chunks of w_oT
    # side by side; b_pack [128, 8] = [b_q, b_k, b_g0..3, b_o0..1] columns.
    x_d = nc.dram_tensor("x", [BPC, C, N], F32, kind="ExternalInput").ap()
    wqk_d = nc.dram_tensor("w_qk", [C, 2 * QK], F32, kind="ExternalInput").ap()
    wvg_d = nc.dram_tensor("w_vg", [C, 2 * HID], F32, kind="ExternalInput").ap()
    wop_d = nc.dram_tensor("w_op", [128, 4 * C], F32, kind="ExternalInput").ap()
    bp_d = nc.dram_tensor("b_pack", [128, 8], F32, kind="ExternalInput").ap()
    bv_d = None
    if with_bv:
        bv_d = nc.dram_tensor("b_v_bc", [128, HID], F32, kind="ExternalInput").ap()
    out_d = nc.dram_tensor("out", [BPC, C, N], F32, kind="ExternalOutput").ap()

    with tile.TileContext(nc) as tc, ExitStack() as ctx:
        consts = ctx.enter_context(tc.tile_pool(name="consts", bufs=1))
        xp = ctx.enter_context(tc.tile_pool(name="xp", bufs=2))

        # x tiles for image 0 up front: its first n-chunk is what the very
        # first matmuls need, so its DMA goes ahead of the big weight loads.
        x0_sb = []
        for c in range(2):
            xt = xp.tile([128, N], F32R, name=f"x0_{c}", tag="x")
            x0_sb.append(xt)
        for c in range(2):
            nc.sync.dma_start(x0_sb[c][:, 0:512], _r(x_d[0, c * 128:(c + 1) * 128, 0:512]))

        # packed weight tiles on the gpsimd (SWDGE) queue, in the order the
        # projections consume them, overlapping the x stream on the sync queue
        wqk_sb = [consts.tile([128, 2 * QK], F32R, name=f"wqk{c}", tag=f"wqk{c}")
                  for c in range(2)]
        for c in range(2):
            nc.gpsimd.dma_start(wqk_sb[c][:], _r(wqk_d[c * 128:(c + 1) * 128, :]))
        bp_sb = consts.tile([128, 8], F32, name="bp", tag="bp")
        nc.gpsimd.dma_start(bp_sb[:], bp_d[:, :])
        wvg_sb = [consts.tile([128, 2 * HID], F32R, name=f"wvg{c}", tag=f"wvg{c}")
                  for c in range(2)]
        for c in range(2):
            nc.gpsimd.dma_start(wvg_sb[c][:, 0:HID], _r(wvg_d[c * 128:(c + 1) * 128, 0:HID]))
        for c in range(2):
            nc.gpsimd.dma_start(wvg_sb[c][:, HID:2 * HID],
                                _r(wvg_d[c * 128:(c + 1) * 128, HID:2 * HID]))
        wop_sb = consts.tile([128, 4 * C], F32R, name="wop", tag="wop")
        nc.gpsimd.dma_start(wop_sb[:], _r(wop_d[:, :]))

        wq_sb = [wqk_sb[c][:, 0:QK] for c in range(2)]
        wk_sb = [wqk_sb[c][:, QK:2 * QK] for c in range(2)]
        wv_sb = [wvg_sb[c][:, 0:HID] for c in range(2)]
        wg_sb = [wvg_sb[c][:, HID:2 * HID] for c in range(2)]
        wo_sb = [wop_sb[:, h * C:(h + 1) * C] for h in range(4)]
        bq_sb = bp_sb[0:QK, 0:1]
        bk_sb = bp_sb[0:QK, 1:2]
        bg_sb = [bp_sb[:, 2 + h:3 + h] for h in range(4)]
        bo_sb = [bp_sb[:, 6 + o:7 + o] for o in range(2)]
        bv_sb = None
        if with_bv:
            bv_sb = consts.tile([128, HID], F32, name="bv", tag="bv")
            nc.sync.dma_start(bv_sb[:], bv_d[:, :])
        qkp = ctx.enter_context(tc.tile_pool(name="qkp", bufs=2))
        vtp = ctx.enter_context(tc.tile_pool(name="vtp", bufs=NJ))
        gp = ctx.enter_context(tc.tile_pool(name="gp", bufs=4))
        atp = ctx.enter_context(tc.tile_pool(name="atp", bufs=3))
        rlp = ctx.enter_context(tc.tile_pool(name="rlp", bufs=3))
        vgp = ctx.enter_context(tc.tile_pool(name="vgp", bufs=8))
        xrp = ctx.enter_context(tc.tile_pool(name="xrp", bufs=4))
        obp = ctx.enter_context(tc.tile_pool(name="obp", bufs=4))
        psp = ctx.enter_context(tc.tile_pool(name="psp", bufs=4, space="PSUM"))
        vpsp = ctx.enter_context(tc.tile_pool(name="vpsp", bufs=4, space="PSUM"))

        for rep in range(reps):
          for img in range(BPC):
              # ---- load x (two 128-channel chunks), split along n so the first
              # projection matmuls can start before the whole image lands ----
              if img == 0 and rep == 0:
                  x_sb = x0_sb
                  chunks = NCH[1:]  # chunk 0 already in flight
              else:
                  x_sb = []
                  for c in range(2):
                      xt = xp.tile([128, N], F32R, name=f"x{img}_{c}", tag="x")
                      x_sb.append(xt)
                  chunks = NCH
              for (n0, S) in chunks:
                  for c in range(2):
                      nc.sync.dma_start(x_sb[c][:, n0:n0 + S],
                                        _r(x_d[img, c * 128:(c + 1) * 128, n0:n0 + S]))

              # ---- projections, interleaved per n-chunk so each arriving x
              # chunk unlocks q/k + vT + gate work immediately ----
              q_sb = qkp.tile([QK, N], F32R, name=f"q{img}", tag="qk")
              k_sb = qkp.tile([QK, N], F32R, name=f"k{img}", tag="qk")
              g_sb = [gp.tile([128, N], F32, name=f"g{img}_{hs}", tag="g")
                      for hs in range(4)]
              vt_sb = [None] * NJ
              for (n0, S) in NCH:
                  # q/k: [96, S] chunks
                  for dst, w_sb, b_sb in ((q_sb, wq_sb, bq_sb), (k_sb, wk_sb, bk_sb)):
                      ps = psp.tile([QK, 512], F32, name="ps_qk", tag="ps")
                      nc.tensor.matmul(ps[:, :S], w_sb[0][:], x_sb[0][:, n0:n0 + S],
                                       start=True, stop=False)
                      nc.tensor.matmul(ps[:, :S], w_sb[1][:], x_sb[1][:, n0:n0 + S],
                                       start=False, stop=True)
                      nc.scalar.activation(dst[:, n0:n0 + S], ps[:, :S], act,
                                           bias=b_sb)
                  # vT: position-rows j covered by this chunk
                  for j in range(n0 // 128, (n0 + S) // 128):
                      ps = psp.tile([128, 512], F32, name="ps_v", tag="ps")
                      nc.tensor.matmul(ps[:], x_sb[0][:, j * 128:(j + 1) * 128],
                                       wv_sb[0][:], start=True, stop=False)
                      nc.tensor.matmul(ps[:], x_sb[1][:, j * 128:(j + 1) * 128],
                                       wv_sb[1][:], start=False, stop=True)
                      vt = vtp.tile([128, HID], F32R, name=f"vt{img}_{j}", tag="vt")
                      if with_bv:
                          nc.vector.tensor_add(ps[:], ps[:], bv_sb[:])
                      nc.scalar.activation(vt[:], ps[:], act)
                      vt_sb[j] = vt
                  # gate: [128, S] x 4 h-chunks
                  for hs in range(4):
                      ps = psp.tile([128, 512], F32, name="ps_g", tag="ps")
                      nc.tensor.matmul(ps[:, :S], wg_sb[0][:, hs * 128:(hs + 1) * 128],
                                       x_sb[0][:, n0:n0 + S], start=True, stop=False)
                      nc.tensor.matmul(ps[:, :S], wg_sb[1][:, hs * 128:(hs + 1) * 128],
                                       x_sb[1][:, n0:n0 + S], start=False, stop=True)
                      nc.scalar.activation(g_sb[hs][:, n0:n0 + S], ps[:, :S], act,
                                           bias=bg_sb[hs])

              # ---- attention + gating + out-projection, per n-chunk.
              # The out-projection/residual of chunk i is emitted inside chunk
              # i+1's j-loop (after its first sim) so the PE fills the gating
              # DVE latency window instead of stalling at the chunk boundary.
              def emit_outproj(n0, S, vg):
                  for os in range(2):
                      ps = psp.tile([128, 512], F32, name="ps_o", tag="ps")
                      for hs in range(4):
                          nc.tensor.matmul(ps[:, :S],
                                           wo_sb[hs][:, os * 128:(os + 1) * 128],
                                           vg[hs][:, :S],
                                           start=(hs == 0), stop=(hs == 3),
                                           skip_group_check=True)
                      xr = xrp.tile([128, 512], F32, name="xr", tag="xr")
                      nc.sync.dma_start(xr[:, :S], x_d[img, os * 128:(os + 1) * 128,
                                                       n0:n0 + S])
                      ob = obp.tile([128, 512], F32, name="ob", tag="ob")
                      # (psum + b_out) + x_residual
                      nc.vector.scalar_tensor_tensor(ob[:, :S], ps[:, :S],
                                                     bo_sb[os], xr[:, :S],
                                                     ALU.add, ALU.add)
                      nc.sync.dma_start(out_d[img, os * 128:(os + 1) * 128, n0:n0 + S],
                                        ob[:, :S])

              pending = None
              for (n0, S) in NCH:
                  vps = [vpsp.tile([128, 512], F32, name=f"vps{hs}", tag="vps")
                         for hs in range(4)]
                  at_prev = None
                  for j in range(NJ):
                      # simT_j = k_j^T @ q_chunk  -> [128 positions(m), S positions(n)]
                      sim = psp.tile([128, 512], F32, name="ps_sim", tag="ps")
                      nc.tensor.matmul(sim[:, :S], k_sb[:, j * 128:(j + 1) * 128],
                                       q_sb[:, n0:n0 + S], start=True, stop=True)
                      # relu on DVE (single PSUM read), square on ACT -> f32r AT
                      rl = rlp.tile([128, 512], F32, name="rl", tag="rl")
                      nc.vector.tensor_scalar_max(rl[:, :S], sim[:, :S], 0.0)
                      at = atp.tile([128, 512], F32R, name="at", tag="at")
                      nc.scalar.square(at[:, :S], rl[:, :S])
                      if j == 1 and pending is not None:
                          emit_outproj(*pending)
                          pending = None
                      if at_prev is not None:
                          jp = j - 1
                          for hs in range(4):
                              nc.tensor.matmul(
                                  vps[hs][:, :S],
                                  vt_sb[jp][:, hs * 128:(hs + 1) * 128],
                                  at_prev[:, :S],
                                  start=(jp == 0), stop=False, skip_group_check=True)
                      at_prev = at
                  for hs in range(4):
                      nc.tensor.matmul(
                          vps[hs][:, :S],
                          vt_sb[NJ - 1][:, hs * 128:(hs + 1) * 128],
                          at_prev[:, :S],
                          start=False, stop=True, skip_group_check=True)

                  # gating right after the last V matmuls
                  vg = []
                  for hs in range(4):
                      vgt = vgp.tile([128, 512], F32R, name="vg", tag="vg")
                      nc.vector.tensor_mul(vgt[:, :S], vps[hs][:, :S],
                                           g_sb[hs][:, n0:n0 + S])
                      vg.append(vgt)
                  pending = (n0, S, vg)
              emit_outproj(*pending)
    nc.compile()
    return nc


_CACHE = {}


def _get_nc(with_bv: bool) -> bass.Bass:
    if with_bv not in _CACHE:
        _CACHE[with_bv] = build_bass(with_bv)
    return _CACHE[with_bv]


def _make_in_maps(inputs: dict):
    x = np.ascontiguousarray(np.asarray(inputs["x"], dtype=np.float32))
    w_hidden = np.asarray(inputs["w_hidden"], dtype=np.float32)
    b_hidden = np.asarray(inputs["b_hidden"], dtype=np.float32)
    w_qk = np.asarray(inputs["w_qk"], dtype=np.float32)
    b_qk = np.asarray(inputs["b_qk"], dtype=np.float32)
    w_out = np.asarray(inputs["w_out"], dtype=np.float32)
    b_out = np.asarray(inputs["b_out"], dtype=np.float32)

    b_v = b_hidden[:HID]
    with_bv = bool(np.any(b_v != 0.0))

    w_oT = w_out.T  # [HID, C]
    w_op = np.ascontiguousarray(
        w_oT.reshape(4, 128, C).transpose(1, 0, 2).reshape(128, 4 * C))
    b_pack = np.zeros((128, 8), np.float32)
    b_pack[:QK, 0] = b_qk[:QK]
    b_pack[:QK, 1] = b_qk[QK:]
    b_pack[:, 2:6] = b_hidden[HID:].reshape(4, 128).T
    b_pack[:, 6:8] = b_out.reshape(2, 128).T
    base = {
        "w_qk": np.ascontiguousarray(w_qk.T),
        "w_vg": np.ascontiguousarray(w_hidden.T),
        "w_op": w_op,
        "b_pack": b_pack,
    }
    if with_bv:
        base["b_v_bc"] = np.ascontiguousarray(np.tile(b_v[None, :], (128, 1)))

    xs = x.reshape(B, C, N)
    in_maps = [
        {**base, "x": np.ascontiguousarray(xs[i * BPC:(i + 1) * BPC])}
        for i in range(NCORES)
    ]
    return in_maps, with_bv


def _run(inputs: dict, trace: bool = False):
    in_maps, with_bv = _make_in_maps(inputs)
    nc = _get_nc(with_bv)
    res = run_bass_kernel_spmd(nc, in_maps, core_ids=list(range(NCORES)),
                               trace=trace)
    out = np.concatenate([res.results[i]["out"] for i in range(NCORES)], axis=0)
    return out.reshape(B, C, 48, 48), res


def kernel(**inputs) -> np.ndarray:
    out, _ = _run(inputs, trace=False)
    return out

